# revision 38
# baseline (speedup 1.0000x reference)
"""Trainium2 Bass kernel for nn_AttentionBlock_15693810500077.

GroupNorm(32 groups) -> 1x1 qkv conv -> 4-head attention (T=4096) ->
1x1 proj -> residual, for x [2, 256, 16, 16, 16] fp32.

Sharding: 8 cores = (batch b in {0,1}) x (t-slice i in {0..3}, TS=1024).
Each core computes the full attention rows for its t-slice of its batch,
for all 4 heads, plus the projection and residual -> y^T slab [1024, 256].
The host rotates each core's x copy (np.roll over T) so the core's t-slice
always sits at columns 0:1024 -> one static SPMD program for all cores
(softmax over keys is permutation invariant).

v3: keeps the PE gap-free so the HAM clock gate stays at 8/8 (2.4 GHz):
- one head at a time (pv accumulator = 2 PSUM banks) with software
  pipelining: PV of iteration sp-1 is emitted between the QK groups of
  iteration sp, so the in-order PE queue never stalls on exp.
- fp8e4 DoubleRow matmuls for qkv and P@V; exp is biased by -2.5 so
  p fits fp8 (bias cancels in the softmax normalize).
- exp on [128,1024] tiles, split between Act (true Exp -> fp8) and DVE
  (Schraudolph: round(s*A+B) -> uint8 = fp8 bits).
- softmax 1/rowsum via Act exp(-ln(rowsum)); rowsum comes free from a
  ones-column in the PV matmul. pv is copied PSUM->SBUF right after the
  accumulation stops so the single pv bank frees for the next head and
  the normalize overlaps the next head's attention.
- v^T production is interleaved into head-0's loop (chunk pair sp+1
  produced during iteration sp).
- x ships as bf16; xn computed on Act+GpSimd straight to fp8; proj bias
  pre-folded into the host-side xT residual slab; QK stays bf16.
"""
import math
import os

import numpy as np

os.environ.setdefault("JAX_COMPILATION_CACHE_DIR", "/tmp/jaxcache")

import concourse.bass as bass
import concourse.tile as tile
from concourse import mybir
from concourse.bass_utils import run_bass_kernel_spmd

F32 = mybir.dt.float32
F32R = mybir.dt.float32r
BF16 = mybir.dt.bfloat16
F8 = mybir.dt.float8e4
U8 = mybir.dt.uint8
AF = mybir.ActivationFunctionType
ALU = mybir.AluOpType
DRM = mybir.MatmulPerfMode.DoubleRow

H = 4
C = 256
T = 4096
TS = 1024
EPS = 1e-5
SCALE2 = 0.125            # (1/sqrt(sqrt(64)))^2, applied inside exp
EBIAS = -2.5              # keeps p <= ~96 < 240 (fp8e4 max); cancels in norm
SCH_A = SCALE2 * 8.0 / math.log(2.0)
SCH_B = (7 * 8 - 0.3) + EBIAS * (8.0 / math.log(2.0))
NSP = 16                  # chunk pairs (32 key chunks of 128)

# exp engine split per (head, chunk): True -> DVE Schraudolph, else Act Exp.
DVE_FRAC = 0.48


def _use_dve(idx):
    if idx >= 4 * 32 - 6:
        return idx % 2 == 0   # split tail chunks so neither engine backlogs
    return (int((idx + 1) * DVE_FRAC) - int(idx * DVE_FRAC)) > 0


def _dedupe_ldweights(m):
    """Drop InstLdweights that reload the stationary already in the PE array
    (consecutive matmuls sharing the same weights AP). The matmul after a
    dropped load inherits its dependencies."""
    for f in m.functions:
        for blk in f.blocks:
            insts = list(blk.instructions)
            prev_sig = None
            dropped = 0
            for idx, ins in enumerate(insts):
                if not isinstance(ins, mybir.InstLdweights):
                    continue
                sig = (repr(ins.ins[0]), str(ins.perf_mode),
                       str(ins.is_transpose), str(ins.tile_position),
                       str(ins.tile_size))
                if (sig == prev_sig and idx + 1 < len(insts)
                        and isinstance(insts[idx + 1], mybir.InstMatmult)):
                    try:
                        insts[idx + 1].merge_dependencies_from(ins)
                    except Exception:
                        pass
                    blk.instructions.remove(ins)
                    dropped += 1
                else:
                    prev_sig = sig


def build_nc():
    nc = bass.Bass()

    xb_d = nc.dram_tensor("xb", [128, 2, T], BF16, kind="ExternalInput")
    xT_d = nc.dram_tensor("xTb", [TS, C], F32, kind="ExternalInput")
    wq_d = nc.dram_tensor("wq2", [128, 2, C], U8, kind="ExternalInput")
    wk_d = nc.dram_tensor("wk2", [128, 2, C], U8, kind="ExternalInput")
    wv_d = nc.dram_tensor("wv2", [128, 2, C], U8, kind="ExternalInput")
    pT_d = nc.dram_tensor("pT8", [128, 2, C], U8, kind="ExternalInput")
    normw_d = nc.dram_tensor("normw", [2, 128, 1], F32, kind="ExternalInput")
    normb_d = nc.dram_tensor("normb", [2, 128, 1], F32, kind="ExternalInput")
    sel_d = nc.dram_tensor("sel", [128, 16], F32, kind="ExternalInput")
    selN_d = nc.dram_tensor("selN", [128, 16], F32, kind="ExternalInput")
    exp_d = nc.dram_tensor("expand", [16, 128], F32, kind="ExternalInput")
    ones_d = nc.dram_tensor("ones", [128, 128], F32R, kind="ExternalInput")
    yT_d = nc.dram_tensor("yT", [TS, C], F32, kind="ExternalOutput")

    import contextlib

    with tile.TileContext(nc) as tc:
        with (
            tc.tile_pool(name="consts", bufs=1) as consts,
            tc.tile_pool(name="gnp", bufs=2) as gnp,
            tc.tile_pool(name="kqv", bufs=1) as kqv,
            tc.tile_pool(name="psA", bufs=2, space="PSUM") as psA,
            tc.tile_pool(name="psPV", bufs=1, space="PSUM") as psPV,
            tc.tile_pool(name="psB", bufs=2, space="PSUM") as psB,
            contextlib.ExitStack() as late,
        ):
            # ---- x first (critical path), weights after, xT deferred ----
            xp = late.enter_context(tc.tile_pool(name="xp", bufs=1))
            xb = xp.tile([128, 2, T], BF16, name="xb")
            for jc in range(4):
                for i in range(2):
                    eng = nc.sync if i == 0 else nc.scalar
                    eng.dma_start(
                        out=xb[:, i, jc * 1024:(jc + 1) * 1024],
                        in_=xb_d[:, i, jc * 1024:(jc + 1) * 1024],
                    )
            wq2 = consts.tile([128, 2, C], U8, name="wq2")
            wk2 = consts.tile([128, 2, C], U8, name="wk2")
            wv2 = consts.tile([128, 2, C], U8, name="wv2")
            nc.sync.dma_start(out=wq2, in_=wq_d[:])
            nc.sync.dma_start(out=wk2, in_=wk_d[:])
            nc.sync.dma_start(out=wv2, in_=wv_d[:])
            pT8 = consts.tile([128, 2, C], U8, name="pT8")
            nc.sync.dma_start(out=pT8, in_=pT_d[:])
            normw = [consts.tile([128, 1], F32, name=f"nw{i}") for i in range(2)]
            normb = [consts.tile([128, 1], F32, name=f"nb{i}") for i in range(2)]
            for i in range(2):
                nc.sync.dma_start(out=normw[i], in_=normw_d[i])
                nc.sync.dma_start(out=normb[i], in_=normb_d[i])
            sel = consts.tile([128, 16], F32, name="sel")
            nc.sync.dma_start(out=sel, in_=sel_d[:])
            selN = consts.tile([128, 16], F32, name="selN")
            nc.sync.dma_start(out=selN, in_=selN_d[:])
            expand = consts.tile([16, 128], F32, name="expand")
            nc.sync.dma_start(out=expand, in_=exp_d[:])
            ones = consts.tile([128, 128], F32R, name="ones")
            nc.sync.dma_start(out=ones, in_=ones_d[:])
            xT_sb = consts.tile([128, 8, C], F32, name="xT_sb")
            ebias = consts.tile([128, 1], F32, name="ebias")
            nc.vector.memset(ebias, EBIAS)

            # ---- late pools ----
            ppool = late.enter_context(tc.tile_pool(name="ppool", bufs=3))
            rsp = late.enter_context(tc.tile_pool(name="rsp", bufs=2))
            stk = late.enter_context(tc.tile_pool(name="stk", bufs=1))
            outp = late.enter_context(tc.tile_pool(name="outp", bufs=1))

            # ---- GroupNorm -> xn fp8, then q/k ----
            xn2 = kqv.tile([128, 2, T], U8, name="xn2")
            xn8 = xn2.bitcast(F8)
            q2 = kqv.tile([128, 2, TS], BF16, name="q2")
            k2 = kqv.tile([128, 2, T], BF16, name="k2")
            ab_sb = {}
            if True:
                # block-1 stats on Act via accum_out (sum + sum of squares),
                # concurrent with the DVE bn_stats of block 0
                scr = gnp.tile([128, T], BF16, name="scr", tag="scr")
                sp4 = gnp.tile([128, 4, 2], F32, name="sp4", tag="sp4")
                for jc in range(4):
                    sl = slice(jc * 1024, (jc + 1) * 1024)
                    nc.scalar.activation(scr[:, sl], xb[:, 1, sl], AF.Square,
                                         accum_out=sp4[:, jc, 1:2])
                    nc.scalar.activation(scr[:, sl], xb[:, 1, sl], AF.Identity,
                                         accum_out=sp4[:, jc, 0:1])
                sp2 = gnp.tile([128, 2, 2], F32, name="sp2", tag="sp2")
                nc.vector.tensor_add(sp2, sp4[:, 0:2, :], sp4[:, 2:4, :])
                ss2 = gnp.tile([128, 2], F32, name="ss2", tag="ss2")
                nc.vector.tensor_add(ss2, sp2[:, 0, :], sp2[:, 1, :])
                ssum = ss2[:, 0:1]
                ssq = ss2[:, 1:2]

                def gn_math(i, gm_in, gx_in):
                    """group stats -> per-partition scale/bias for block i."""
                    gm_sb = gnp.tile([16, 1], F32, name="gm_sb", tag="gm_sb")
                    nc.vector.tensor_copy(gm_sb, gm_in)
                    gmsq = gnp.tile([16, 1], F32, name="gmsq", tag="gmsq")
                    nc.vector.tensor_mul(gmsq, gm_sb, gm_sb)
                    gvar = gnp.tile([16, 1], F32, name="gvar", tag="gvar")
                    nc.vector.scalar_tensor_tensor(
                        gvar, gx_in, EPS, gmsq, op0=ALU.add, op1=ALU.subtract
                    )
                    lnv = gnp.tile([16, 1], F32, name="lnv", tag="lnv")
                    nc.scalar.activation(lnv, gvar, AF.Ln)
                    rstd = gnp.tile([16, 1], F32, name="rstd", tag="rstd")
                    nc.scalar.activation(rstd, lnv, AF.Exp, scale=-0.5)
                    me_ps = psB.tile([128, 1], F32, name="me_ps", tag="vt")
                    nc.tensor.matmul(me_ps, expand, gm_sb, start=True, stop=True)
                    re_ps = psB.tile([128, 1], F32, name="re_ps", tag="vt")
                    nc.tensor.matmul(re_ps, expand, rstd, start=True, stop=True)
                    a_sb = gnp.tile([128, 1], F32, name="a_sb", tag=f"a_sb{i}")
                    nc.vector.tensor_mul(a_sb, re_ps, normw[i])
                    t2 = gnp.tile([128, 1], F32, name="t2", tag="t2")
                    nc.vector.tensor_mul(t2, me_ps, a_sb)
                    b_sb = gnp.tile([128, 1], F32, name="b_sb", tag=f"b_sb{i}")
                    nc.vector.tensor_sub(b_sb, normb[i], t2)
                    ab_sb[i] = (a_sb, b_sb)

                # block 0: DVE bn_stats path
                xv = xb[:, 0, :].rearrange("p (a f) -> p a f", f=512)
                stats = gnp.tile([128, 8, 6], F32, name="stats", tag="stats")
                for j in range(8):
                    nc.vector.bn_stats(out=stats[:, j, :], in_=xv[:, j, :])
                mv = gnp.tile([128, 2], F32, name="mv", tag="mv")
                nc.vector.bn_aggr(out=mv, in_=stats)
                msq = gnp.tile([128, 1], F32, name="msq", tag="msq")
                nc.vector.tensor_mul(msq, mv[:, 0:1], mv[:, 0:1])
                exsq = gnp.tile([128, 1], F32, name="exsq", tag="exsq")
                nc.vector.tensor_add(exsq, msq, mv[:, 1:2])
                gm_ps = psB.tile([16, 1], F32, name="gm_ps", tag="vt")
                nc.tensor.matmul(gm_ps, sel, mv[:, 0:1], start=True, stop=True)
                gx_ps = psB.tile([16, 1], F32, name="gx_ps", tag="vt")
                nc.tensor.matmul(gx_ps, sel, exsq, start=True, stop=True)
                gn_math(0, gm_ps, gx_ps)
                # block 1: group stats straight from the Act raw sums
                gm_ps1 = psB.tile([16, 1], F32, name="gm_ps1", tag="vt")
                nc.tensor.matmul(gm_ps1, selN, ssum, start=True, stop=True)
                gx_ps1 = psB.tile([16, 1], F32, name="gx_ps1", tag="vt")
                nc.tensor.matmul(gx_ps1, selN, ssq, start=True, stop=True)
                gn_math(1, gm_ps1, gx_ps1)

                # xn: Act cols 0:2048 of both blocks (q + first k chunks),
                # DVE block-0 tail, GpSimd block-1 tail
                nc.scalar.activation(
                    xn2[:, 0, 0:2048].bitcast(F8), xb[:, 0, 0:2048],
                    AF.Identity, bias=ab_sb[0][1], scale=ab_sb[0][0],
                )
                nc.scalar.activation(
                    xn2[:, 1, 0:2048].bitcast(F8), xb[:, 1, 0:2048],
                    AF.Identity, bias=ab_sb[1][1], scale=ab_sb[1][0],
                )
                nc.vector.tensor_scalar(
                    out=xn2[:, 0, 2048:T].bitcast(F8), in0=xb[:, 0, 2048:T],
                    scalar1=ab_sb[0][0], scalar2=ab_sb[0][1],
                    op0=ALU.mult, op1=ALU.add,
                )
                nc.gpsimd.tensor_scalar(
                    out=xn2[:, 1, 2048:T].bitcast(F8), in0=xb[:, 1, 2048:T],
                    scalar1=ab_sb[1][0], scalar2=ab_sb[1][1],
                    op0=ALU.mult, op1=ALU.add,
                )
                for o in range(2):
                    q_ps = psA.tile([128, TS], F32, name="q_ps", tag="big")
                    for nn in range(2):
                        sl = slice(nn * 512, (nn + 1) * 512)
                        nc.tensor.matmul(
                            q_ps[:, sl],
                            wq2.bitcast(F8)[:, :, o * 128:(o + 1) * 128],
                            xn8[:, :, sl], start=True, stop=True,
                            perf_mode=DRM,
                        )
                    nc.vector.tensor_copy(q2[:, o, :], q_ps)
                def make_k(o, nkp):
                    k_ps = psA.tile([128, TS], F32, name="k_ps", tag="big")
                    for nn in range(2):
                        nk = nkp * 2 + nn
                        sl = slice(nk * 512, (nk + 1) * 512)
                        nc.tensor.matmul(
                            k_ps[:, nn * 512:(nn + 1) * 512],
                            wk2.bitcast(F8)[:, :, o * 128:(o + 1) * 128],
                            xn8[:, :, sl], start=True, stop=True,
                            perf_mode=DRM,
                        )
                    sl2 = slice(nkp * 1024, (nkp + 1) * 1024)
                    if nkp % 2 == 0:
                        nc.vector.tensor_copy(k2[:, o, sl2], k_ps)
                    else:
                        nc.scalar.copy(k2[:, o, sl2], k_ps)

                # only the o=0 block (heads 0/1) up-front; o=1 is produced
                # lazily inside head 0's attention pass
                for nkp in range(4):
                    make_k(0, nkp)

            vT2 = kqv.tile([128, H, NSP, 2, 80], U8, name="vT2")
            nc.vector.memset(vT2[:, :, :, :, 64:65], 0x38)  # fp8e4 1.0 bits

            def make_v(pair_i):
                """Produce v^T chunk pair pair_i (two 128-key chunks)."""
                for half in range(2):
                    tci = pair_i * 2 + half
                    vt_ps = psB.tile([128, C], F32, name="vt_ps", tag="vt")
                    nc.tensor.matmul(
                        vt_ps, xn8[:, :, tci * 128:(tci + 1) * 128],
                        wv2.bitcast(F8), start=True, stop=True, perf_mode=DRM,
                    )
                    dst = vT2[:, :, pair_i, half, 0:64].bitcast(F8)
                    src = vt_ps.rearrange("p (h c) -> p h c", h=H)
                    if half == 0:
                        nc.vector.tensor_copy(dst, src)
                    else:
                        nc.scalar.copy(dst, src)

            make_v(0)
            make_v(1)
            # residual slab, needed only by the tail projection
            nc.sync.dma_start(
                out=xT_sb, in_=xT_d.rearrange("(a p) o -> p a o", p=128)
            )

            # ---- attention: one head per pass, PV pipelined one sp behind;
            # normalize's PE part is deferred into the next head's loop ----
            stack4 = stk.tile([128, 2, TS], U8, name="stack4")
            ei = 0
            pending_bc = []

            def norm_pe(h, pvs, recip):
                lo2 = (h % 2) * 64
                for qh in range(2):
                    qs = slice(qh * 512, (qh + 1) * 512)
                    bc_big = psA.tile([128, TS], F32, name="bc", tag="big")
                    nc.tensor.matmul(bc_big[0:64, 0:512], ones[0:1, 0:64],
                                     recip[:, qs], start=True, stop=True)
                    nc.vector.tensor_mul(
                        stack4[lo2:lo2 + 64, h // 2, qs].bitcast(F8),
                        pvs[0:64, qs], bc_big[0:64, 0:512])

            for h in range(H):
                o, lo = h // 2, (h % 2) * 64
                pv_ps = psPV.tile([65, TS], F32, name=f"pv{h}", tag="pv")
                p_hist = []
                for sp in range(NSP):
                    p2 = ppool.tile([128, 2, TS], U8, name="p2", tag="p")
                    for half in range(2):
                        sc = sp * 2 + half
                        kt = k2[lo:lo + 64, o, sc * 128:(sc + 1) * 128]
                        qk_ps = psA.tile([128, TS], F32, name="qk_ps", tag="big")
                        for qh in range(2):
                            qs = slice(qh * 512, (qh + 1) * 512)
                            nc.tensor.matmul(
                                qk_ps[:, qs], kt, q2[lo:lo + 64, o, qs],
                                start=True, stop=True,
                            )
                        if _use_dve(ei):
                            nc.vector.tensor_scalar(
                                out=p2[:, half, :], in0=qk_ps,
                                scalar1=SCH_A, scalar2=SCH_B,
                                op0=ALU.mult, op1=ALU.add,
                            )
                        else:
                            nc.scalar.activation(
                                p2[:, half, :].bitcast(F8), qk_ps,
                                AF.Exp, scale=SCALE2, bias=ebias,
                            )
                        ei += 1
                    p_hist.append(p2)
                    # lazily produce v chunk pair sp+2 and the o=1 k block
                    # during head 0 (heads 2/3 need it much later)
                    if h == 0 and sp + 2 < NSP:
                        make_v(sp + 2)
                    if h == 0 and sp in (1, 5, 9, 13):
                        make_k(1, (sp - 1) // 4)
                    # deferred normalize (PE part) of the previous head
                    if sp == 2 and pending_bc:
                        norm_pe(*pending_bc.pop())
                    # PV for iteration sp-1 (its exps are long done)
                    if sp > 0:
                        pprev = p_hist[sp - 1]
                        for qh in range(2):
                            qs = slice(qh * 512, (qh + 1) * 512)
                            nc.tensor.matmul(
                                pv_ps[:, qs],
                                vT2[:, h, sp - 1, :, 0:65].bitcast(F8),
                                pprev.bitcast(F8)[:, :, qs],
                                start=(sp == 1), stop=False,
                                perf_mode=DRM,
                            )
                for qh in range(2):
                    qs = slice(qh * 512, (qh + 1) * 512)
                    nc.tensor.matmul(
                        pv_ps[:, qs],
                        vT2[:, h, NSP - 1, :, 0:65].bitcast(F8),
                        p_hist[NSP - 1].bitcast(F8)[:, :, qs],
                        start=False, stop=True, perf_mode=DRM,
                    )
                # free the pv bank fast; Act computes 1/rowsum = exp(-ln)
                # straight from PSUM; the PE broadcast + DVE multiply run
                # inside the next head's pass
                lnr = rsp.tile([1, TS], F32, name="lnr", tag="lnr")
                nc.scalar.activation(lnr, pv_ps[64:65, :], AF.Ln)
                pvs = rsp.tile([65, TS], F32, name="pvs", tag="pvs")
                nc.vector.tensor_copy(pvs, pv_ps)
                recip = rsp.tile([1, TS], F32R, name="recip", tag="recip")
                nc.scalar.activation(recip, lnr, AF.Exp, scale=-1.0)
                pending_bc.append((h, pvs, recip))

            while pending_bc:
                norm_pe(*pending_bc.pop())

            # ---- proj (fp8 DoubleRow over head pairs) + residual ----
            out_sb = outp.tile([128, 8, C], F32, name="out_sb")
            for tci in range(8):
                pr_ps = psB.tile([128, C], F32, name="pr_ps", tag="vt")
                nc.tensor.matmul(
                    pr_ps, stack4[:, :, tci * 128:(tci + 1) * 128].bitcast(F8),
                    pT8.bitcast(F8), start=True, stop=True, perf_mode=DRM,
                )
                nc.vector.tensor_add(out_sb[:, tci, :], pr_ps, xT_sb[:, tci, :])
                eng = nc.sync if tci % 2 == 0 else nc.scalar
                eng.dma_start(
                    out=yT_d[tci * 128:(tci + 1) * 128, :], in_=out_sb[:, tci, :]
                )

    import bass_rust as _bass_rust
    _bass_rust.move_matmul_waits_to_ldweights(nc.m)
    _bass_rust.generate_event_semaphores(nc)
    return nc


def host_prep(inputs):
    """Per-core input dicts (slicing / transpose / dtype packing only)."""
    import ml_dtypes
    bf = ml_dtypes.bfloat16
    f8 = ml_dtypes.float8_e4m3

    x = np.ascontiguousarray(np.asarray(inputs["x"], np.float32).reshape(2, C, T))
    qkv_w = np.asarray(inputs["qkv_w"], np.float32)
    proj_w = np.asarray(inputs["proj_w"], np.float32)
    norm_w = np.ascontiguousarray(np.asarray(inputs["norm_w"], np.float32))
    norm_b = np.ascontiguousarray(np.asarray(inputs["norm_b"], np.float32))
    proj_b = np.ascontiguousarray(np.asarray(inputs["proj_b"], np.float32))

    q_idx = np.concatenate([np.arange(h * 192, h * 192 + 64) for h in range(H)])
    wqT = qkv_w[q_idx].T
    wkT = qkv_w[q_idx + 64].T
    wvT = qkv_w[q_idx + 128].T

    def dr_pack(wT):
        return np.ascontiguousarray(
            wT.reshape(2, 128, C).transpose(1, 0, 2).astype(f8)).view(np.uint8)

    pT8 = dr_pack(proj_w.T)

    sel = np.zeros((128, 16), np.float32)
    sel[np.arange(128), np.arange(128) // 8] = 1.0 / 8.0
    selN = sel / 4096.0
    expand = np.zeros((16, 128), np.float32)
    expand[np.arange(128) // 8, np.arange(128)] = 1.0

    shared = {
        "wq2": dr_pack(wqT), "wk2": dr_pack(wkT), "wv2": dr_pack(wvT),
        "pT8": pT8,
        "normw": np.ascontiguousarray(norm_w.reshape(2, 128, 1)),
        "normb": np.ascontiguousarray(norm_b.reshape(2, 128, 1)),
        "sel": sel, "selN": selN, "expand": expand,
        "ones": np.ones((128, 128), np.float32),
    }
    in_maps = []
    for core in range(8):
        b, i = core // 4, core % 4
        t0 = i * TS
        m = dict(shared)
        xr = np.roll(x[b], -t0, axis=1)
        m["xb"] = np.ascontiguousarray(xr.reshape(2, 128, T).transpose(1, 0, 2)
                                       ).astype(bf)
        m["xTb"] = np.ascontiguousarray(x[b, :, t0:t0 + TS].T
                                        + proj_b[None, :])
        in_maps.append(m)
    return in_maps


def gather(core_outs):
    y = np.empty((2, C, T), np.float32)
    for core in range(8):
        b, i = core // 4, core % 4
        y[b, :, i * TS:(i + 1) * TS] = core_outs[core].T
    return y.reshape(2, C, 16, 16, 16)


_NC = None


def _get_nc():
    global _NC
    if _NC is None:
        _NC = build_nc()
    return _NC


def run(inputs, trace=False, trace_cores=None):
    nc = _get_nc()
    in_maps = host_prep(inputs)
    res = run_bass_kernel_spmd(
        nc, in_maps, list(range(8)), trace=trace, trace_cores=trace_cores
    )
    out = gather([res.results[c]["yT"] for c in range(8)])
    return out, res


def kernel(**inputs) -> np.ndarray:
    out, _ = run(inputs)
    return out


# revision 40
# speedup vs baseline: 1.0035x; 1.0035x over previous
"""Trainium2 Bass kernel for nn_AttentionBlock_15693810500077.

GroupNorm(32 groups) -> 1x1 qkv conv -> 4-head attention (T=4096) ->
1x1 proj -> residual, for x [2, 256, 16, 16, 16] fp32.

Sharding: 8 cores = (batch b in {0,1}) x (t-slice i in {0..3}, TS=1024).
Each core computes the full attention rows for its t-slice of its batch,
for all 4 heads, plus the projection and residual -> y^T slab [1024, 256].
The host rotates each core's x copy (np.roll over T) so the core's t-slice
always sits at columns 0:1024 -> one static SPMD program for all cores
(softmax over keys is permutation invariant).

v3: keeps the PE gap-free so the HAM clock gate stays at 8/8 (2.4 GHz):
- one head at a time (pv accumulator = 2 PSUM banks) with software
  pipelining: PV of iteration sp-1 is emitted between the QK groups of
  iteration sp, so the in-order PE queue never stalls on exp.
- fp8e4 DoubleRow matmuls for qkv and P@V; exp is biased by -2.5 so
  p fits fp8 (bias cancels in the softmax normalize).
- exp on [128,1024] tiles, split between Act (true Exp -> fp8) and DVE
  (Schraudolph: round(s*A+B) -> uint8 = fp8 bits).
- softmax 1/rowsum via Act exp(-ln(rowsum)); rowsum comes free from a
  ones-column in the PV matmul. pv is copied PSUM->SBUF right after the
  accumulation stops so the single pv bank frees for the next head and
  the normalize overlaps the next head's attention.
- v^T production is interleaved into head-0's loop (chunk pair sp+1
  produced during iteration sp).
- x ships as bf16; xn computed on Act+GpSimd straight to fp8; proj bias
  pre-folded into the host-side xT residual slab; QK stays bf16.
"""
import math
import os

import numpy as np

os.environ.setdefault("JAX_COMPILATION_CACHE_DIR", "/tmp/jaxcache")

import concourse.bass as bass
import concourse.tile as tile
from concourse import mybir
from concourse.bass_utils import run_bass_kernel_spmd

F32 = mybir.dt.float32
F32R = mybir.dt.float32r
BF16 = mybir.dt.bfloat16
F8 = mybir.dt.float8e4
U8 = mybir.dt.uint8
AF = mybir.ActivationFunctionType
ALU = mybir.AluOpType
DRM = mybir.MatmulPerfMode.DoubleRow

H = 4
C = 256
T = 4096
TS = 1024
EPS = 1e-5
SCALE2 = 0.125            # (1/sqrt(sqrt(64)))^2, applied inside exp
EBIAS = -2.5              # keeps p <= ~96 < 240 (fp8e4 max); cancels in norm
SCH_A = SCALE2 * 8.0 / math.log(2.0)
SCH_B = (7 * 8 - 0.3) + EBIAS * (8.0 / math.log(2.0))
NSP = 16                  # chunk pairs (32 key chunks of 128)

# exp engine split per (head, chunk): True -> DVE Schraudolph, else Act Exp.
DVE_FRAC = 0.48


def _use_dve(idx):
    if idx >= 4 * 32 - 6:
        return idx % 2 == 0   # split tail chunks so neither engine backlogs
    return (int((idx + 1) * DVE_FRAC) - int(idx * DVE_FRAC)) > 0


def _dedupe_ldweights(m):
    """Drop InstLdweights that reload the stationary already in the PE array
    (consecutive matmuls sharing the same weights AP). The matmul after a
    dropped load inherits its dependencies."""
    for f in m.functions:
        for blk in f.blocks:
            insts = list(blk.instructions)
            prev_sig = None
            dropped = 0
            for idx, ins in enumerate(insts):
                if not isinstance(ins, mybir.InstLdweights):
                    continue
                sig = (repr(ins.ins[0]), str(ins.perf_mode),
                       str(ins.is_transpose), str(ins.tile_position),
                       str(ins.tile_size))
                if (sig == prev_sig and idx + 1 < len(insts)
                        and isinstance(insts[idx + 1], mybir.InstMatmult)):
                    try:
                        insts[idx + 1].merge_dependencies_from(ins)
                    except Exception:
                        pass
                    blk.instructions.remove(ins)
                    dropped += 1
                else:
                    prev_sig = sig


def build_nc():
    nc = bass.Bass()

    xb_d = nc.dram_tensor("xb", [128, 2, T], BF16, kind="ExternalInput")
    xT_d = nc.dram_tensor("xTb", [TS, C], F32, kind="ExternalInput")
    wq_d = nc.dram_tensor("wq2", [128, 2, C], U8, kind="ExternalInput")
    wk_d = nc.dram_tensor("wk2", [128, 2, C], U8, kind="ExternalInput")
    wv_d = nc.dram_tensor("wv2", [128, 2, C], U8, kind="ExternalInput")
    pT_d = nc.dram_tensor("pT8", [128, 2, C], U8, kind="ExternalInput")
    normw_d = nc.dram_tensor("normw", [2, 128, 1], F32, kind="ExternalInput")
    normb_d = nc.dram_tensor("normb", [2, 128, 1], F32, kind="ExternalInput")
    sel_d = nc.dram_tensor("sel", [128, 16], F32, kind="ExternalInput")
    selN_d = nc.dram_tensor("selN", [128, 16], F32, kind="ExternalInput")
    exp_d = nc.dram_tensor("expand", [16, 128], F32, kind="ExternalInput")
    ones_d = nc.dram_tensor("ones", [128, 128], F32R, kind="ExternalInput")
    yT_d = nc.dram_tensor("yT", [TS, C], F32, kind="ExternalOutput")

    import contextlib

    with tile.TileContext(nc) as tc:
        with (
            tc.tile_pool(name="consts", bufs=1) as consts,
            tc.tile_pool(name="gnp", bufs=2) as gnp,
            tc.tile_pool(name="kqv", bufs=1) as kqv,
            tc.tile_pool(name="psA", bufs=2, space="PSUM") as psA,
            tc.tile_pool(name="psPV", bufs=1, space="PSUM") as psPV,
            tc.tile_pool(name="psB", bufs=2, space="PSUM") as psB,
            contextlib.ExitStack() as late,
        ):
            # ---- x first (critical path), weights after, xT deferred ----
            xp = late.enter_context(tc.tile_pool(name="xp", bufs=1))
            xb = xp.tile([128, 2, T], BF16, name="xb")
            for jc in range(4):
                for i in range(2):
                    eng = nc.sync if i == 0 else nc.scalar
                    eng.dma_start(
                        out=xb[:, i, jc * 1024:(jc + 1) * 1024],
                        in_=xb_d[:, i, jc * 1024:(jc + 1) * 1024],
                    )
            wq2 = consts.tile([128, 2, C], U8, name="wq2")
            wk2 = consts.tile([128, 2, C], U8, name="wk2")
            wv2 = consts.tile([128, 2, C], U8, name="wv2")
            nc.sync.dma_start(out=wq2, in_=wq_d[:])
            nc.sync.dma_start(out=wk2, in_=wk_d[:])
            nc.sync.dma_start(out=wv2, in_=wv_d[:])
            pT8 = consts.tile([128, 2, C], U8, name="pT8")
            nc.sync.dma_start(out=pT8, in_=pT_d[:])
            normw = [consts.tile([128, 1], F32, name=f"nw{i}") for i in range(2)]
            normb = [consts.tile([128, 1], F32, name=f"nb{i}") for i in range(2)]
            for i in range(2):
                nc.sync.dma_start(out=normw[i], in_=normw_d[i])
                nc.sync.dma_start(out=normb[i], in_=normb_d[i])
            sel = consts.tile([128, 16], F32, name="sel")
            nc.sync.dma_start(out=sel, in_=sel_d[:])
            selN = consts.tile([128, 16], F32, name="selN")
            nc.sync.dma_start(out=selN, in_=selN_d[:])
            expand = consts.tile([16, 128], F32, name="expand")
            nc.sync.dma_start(out=expand, in_=exp_d[:])
            ones = consts.tile([128, 128], F32R, name="ones")
            nc.sync.dma_start(out=ones, in_=ones_d[:])
            xT_sb = consts.tile([128, 8, C], F32, name="xT_sb")
            ebias = consts.tile([128, 1], F32, name="ebias")
            nc.vector.memset(ebias, EBIAS)

            # ---- late pools ----
            ppool = late.enter_context(tc.tile_pool(name="ppool", bufs=3))
            rsp = late.enter_context(tc.tile_pool(name="rsp", bufs=2))
            stk = late.enter_context(tc.tile_pool(name="stk", bufs=1))
            outp = late.enter_context(tc.tile_pool(name="outp", bufs=1))

            # ---- GroupNorm -> xn fp8, then q/k ----
            xn2 = kqv.tile([128, 2, T], U8, name="xn2")
            xn8 = xn2.bitcast(F8)
            q2 = kqv.tile([128, 2, TS], BF16, name="q2")
            k2 = kqv.tile([128, 2, T], BF16, name="k2")
            ab_sb = {}
            if True:
                # block-1 stats on Act via accum_out (sum + sum of squares),
                # concurrent with the DVE bn_stats of block 0
                scr = gnp.tile([128, T], BF16, name="scr", tag="scr")
                sp4 = gnp.tile([128, 4, 2], F32, name="sp4", tag="sp4")
                for jc in range(4):
                    sl = slice(jc * 1024, (jc + 1) * 1024)
                    nc.scalar.activation(scr[:, sl], xb[:, 1, sl], AF.Square,
                                         accum_out=sp4[:, jc, 1:2])
                    nc.scalar.activation(scr[:, sl], xb[:, 1, sl], AF.Identity,
                                         accum_out=sp4[:, jc, 0:1])
                sp2 = gnp.tile([128, 2, 2], F32, name="sp2", tag="sp2")
                nc.vector.tensor_add(sp2, sp4[:, 0:2, :], sp4[:, 2:4, :])
                ss2 = gnp.tile([128, 2], F32, name="ss2", tag="ss2")
                nc.vector.tensor_add(ss2, sp2[:, 0, :], sp2[:, 1, :])
                ssum = ss2[:, 0:1]
                ssq = ss2[:, 1:2]

                def gn_math(i, gm_in, gx_in):
                    """group stats -> per-partition scale/bias for block i."""
                    gm_sb = gnp.tile([16, 1], F32, name="gm_sb", tag="gm_sb")
                    nc.vector.tensor_copy(gm_sb, gm_in)
                    gmsq = gnp.tile([16, 1], F32, name="gmsq", tag="gmsq")
                    nc.vector.tensor_mul(gmsq, gm_sb, gm_sb)
                    gvar = gnp.tile([16, 1], F32, name="gvar", tag="gvar")
                    nc.vector.scalar_tensor_tensor(
                        gvar, gx_in, EPS, gmsq, op0=ALU.add, op1=ALU.subtract
                    )
                    lnv = gnp.tile([16, 1], F32, name="lnv", tag="lnv")
                    nc.scalar.activation(lnv, gvar, AF.Ln)
                    rstd = gnp.tile([16, 1], F32, name="rstd", tag="rstd")
                    nc.scalar.activation(rstd, lnv, AF.Exp, scale=-0.5)
                    me_ps = psB.tile([128, 1], F32, name="me_ps", tag="vt")
                    nc.tensor.matmul(me_ps, expand, gm_sb, start=True, stop=True)
                    re_ps = psB.tile([128, 1], F32, name="re_ps", tag="vt")
                    nc.tensor.matmul(re_ps, expand, rstd, start=True, stop=True)
                    a_sb = gnp.tile([128, 1], F32, name="a_sb", tag=f"a_sb{i}")
                    nc.vector.tensor_mul(a_sb, re_ps, normw[i])
                    t2 = gnp.tile([128, 1], F32, name="t2", tag="t2")
                    nc.vector.tensor_mul(t2, me_ps, a_sb)
                    b_sb = gnp.tile([128, 1], F32, name="b_sb", tag=f"b_sb{i}")
                    nc.vector.tensor_sub(b_sb, normb[i], t2)
                    ab_sb[i] = (a_sb, b_sb)

                # block 0: DVE bn_stats path
                xv = xb[:, 0, :].rearrange("p (a f) -> p a f", f=512)
                stats = gnp.tile([128, 8, 6], F32, name="stats", tag="stats")
                for j in range(8):
                    nc.vector.bn_stats(out=stats[:, j, :], in_=xv[:, j, :])
                mv = gnp.tile([128, 2], F32, name="mv", tag="mv")
                nc.vector.bn_aggr(out=mv, in_=stats)
                msq = gnp.tile([128, 1], F32, name="msq", tag="msq")
                nc.vector.tensor_mul(msq, mv[:, 0:1], mv[:, 0:1])
                exsq = gnp.tile([128, 1], F32, name="exsq", tag="exsq")
                nc.vector.tensor_add(exsq, msq, mv[:, 1:2])
                gm_ps = psB.tile([16, 1], F32, name="gm_ps", tag="vt")
                nc.tensor.matmul(gm_ps, sel, mv[:, 0:1], start=True, stop=True)
                gx_ps = psB.tile([16, 1], F32, name="gx_ps", tag="vt")
                nc.tensor.matmul(gx_ps, sel, exsq, start=True, stop=True)
                gn_math(0, gm_ps, gx_ps)
                # block 1: group stats straight from the Act raw sums
                gm_ps1 = psB.tile([16, 1], F32, name="gm_ps1", tag="vt")
                nc.tensor.matmul(gm_ps1, selN, ssum, start=True, stop=True)
                gx_ps1 = psB.tile([16, 1], F32, name="gx_ps1", tag="vt")
                nc.tensor.matmul(gx_ps1, selN, ssq, start=True, stop=True)
                gn_math(1, gm_ps1, gx_ps1)

                # xn: Act cols 0:2048 of both blocks (q + first k chunks),
                # DVE block-0 tail, GpSimd block-1 tail
                nc.scalar.activation(
                    xn2[:, 0, 0:2048].bitcast(F8), xb[:, 0, 0:2048],
                    AF.Identity, bias=ab_sb[0][1], scale=ab_sb[0][0],
                )
                nc.scalar.activation(
                    xn2[:, 1, 0:2048].bitcast(F8), xb[:, 1, 0:2048],
                    AF.Identity, bias=ab_sb[1][1], scale=ab_sb[1][0],
                )
                nc.vector.tensor_scalar(
                    out=xn2[:, 0, 2048:T].bitcast(F8), in0=xb[:, 0, 2048:T],
                    scalar1=ab_sb[0][0], scalar2=ab_sb[0][1],
                    op0=ALU.mult, op1=ALU.add,
                )
                nc.gpsimd.tensor_scalar(
                    out=xn2[:, 1, 2048:T].bitcast(F8), in0=xb[:, 1, 2048:T],
                    scalar1=ab_sb[1][0], scalar2=ab_sb[1][1],
                    op0=ALU.mult, op1=ALU.add,
                )
                for o in range(2):
                    q_ps = psA.tile([128, TS], F32, name="q_ps", tag="big")
                    for nn in range(2):
                        sl = slice(nn * 512, (nn + 1) * 512)
                        nc.tensor.matmul(
                            q_ps[:, sl],
                            wq2.bitcast(F8)[:, :, o * 128:(o + 1) * 128],
                            xn8[:, :, sl], start=True, stop=True,
                            perf_mode=DRM,
                        )
                    nc.vector.tensor_copy(q2[:, o, :], q_ps)
                def make_k(o, nkp):
                    k_ps = psA.tile([128, TS], F32, name="k_ps", tag="big")
                    for nn in range(2):
                        nk = nkp * 2 + nn
                        sl = slice(nk * 512, (nk + 1) * 512)
                        nc.tensor.matmul(
                            k_ps[:, nn * 512:(nn + 1) * 512],
                            wk2.bitcast(F8)[:, :, o * 128:(o + 1) * 128],
                            xn8[:, :, sl], start=True, stop=True,
                            perf_mode=DRM,
                        )
                    sl2 = slice(nkp * 1024, (nkp + 1) * 1024)
                    if nkp % 2 == 0:
                        nc.vector.tensor_copy(k2[:, o, sl2], k_ps)
                    else:
                        nc.scalar.copy(k2[:, o, sl2], k_ps)

                # only the o=0 block (heads 0/1) up-front; o=1 is produced
                # lazily inside head 0's attention pass
                for nkp in range(4):
                    make_k(0, nkp)

            vT2 = kqv.tile([128, H, NSP, 2, 80], U8, name="vT2")
            nc.vector.memset(vT2[:, :, :, :, 64:65], 0x38)  # fp8e4 1.0 bits

            def make_v(pair_i):
                """Produce v^T chunk pair pair_i (two 128-key chunks)."""
                for half in range(2):
                    tci = pair_i * 2 + half
                    vt_ps = psB.tile([128, C], F32, name="vt_ps", tag="vt")
                    nc.tensor.matmul(
                        vt_ps, xn8[:, :, tci * 128:(tci + 1) * 128],
                        wv2.bitcast(F8), start=True, stop=True, perf_mode=DRM,
                    )
                    dst = vT2[:, :, pair_i, half, 0:64].bitcast(F8)
                    src = vt_ps.rearrange("p (h c) -> p h c", h=H)
                    if half == 0:
                        nc.vector.tensor_copy(dst, src)
                    else:
                        nc.scalar.copy(dst, src)

            make_v(0)
            make_v(1)
            # residual slab, needed only by the tail projection
            nc.sync.dma_start(
                out=xT_sb, in_=xT_d.rearrange("(a p) o -> p a o", p=128)
            )

            # ---- attention: one head per pass, PV pipelined one sp behind;
            # normalize's PE part is deferred into the next head's loop ----
            stack4 = stk.tile([128, 2, TS], U8, name="stack4")
            ei = 0
            pending_bc = []

            def norm_pe(h, pvs, recip):
                lo2 = (h % 2) * 64
                for qh in range(2):
                    qs = slice(qh * 512, (qh + 1) * 512)
                    bc_big = psA.tile([128, TS], F32, name="bc", tag="big")
                    nc.tensor.matmul(bc_big[0:64, 0:512], ones[0:1, 0:64],
                                     recip[:, qs], start=True, stop=True)
                    nc.vector.tensor_mul(
                        stack4[lo2:lo2 + 64, h // 2, qs].bitcast(F8),
                        pvs[0:64, qs], bc_big[0:64, 0:512])

            for h in range(H):
                o, lo = h // 2, (h % 2) * 64
                pv_ps = psPV.tile([65, TS], F32, name=f"pv{h}", tag="pv")
                p_hist = []
                for sp in range(NSP):
                    p2 = ppool.tile([128, 2, TS], U8, name="p2", tag="p")
                    for half in range(2):
                        sc = sp * 2 + half
                        kt = k2[lo:lo + 64, o, sc * 128:(sc + 1) * 128]
                        qk_ps = psA.tile([128, TS], F32, name="qk_ps", tag="big")
                        for qh in range(2):
                            qs = slice(qh * 512, (qh + 1) * 512)
                            nc.tensor.matmul(
                                qk_ps[:, qs], kt, q2[lo:lo + 64, o, qs],
                                start=True, stop=True,
                            )
                        if _use_dve(ei):
                            nc.vector.tensor_scalar(
                                out=p2[:, half, :], in0=qk_ps,
                                scalar1=SCH_A, scalar2=SCH_B,
                                op0=ALU.mult, op1=ALU.add,
                            )
                        else:
                            nc.scalar.activation(
                                p2[:, half, :].bitcast(F8), qk_ps,
                                AF.Exp, scale=SCALE2, bias=ebias,
                            )
                        ei += 1
                    p_hist.append(p2)
                    # lazily produce v chunk pair sp+2 and the o=1 k block
                    # during head 0 (heads 2/3 need it much later)
                    if h == 0 and sp + 2 < NSP:
                        make_v(sp + 2)
                    if h == 0 and sp in (1, 5, 9, 13):
                        make_k(1, (sp - 1) // 4)
                    # deferred normalize (PE part) of the previous head
                    if sp == 2 and pending_bc:
                        norm_pe(*pending_bc.pop())
                    # PV for iteration sp-1 (its exps are long done)
                    if sp > 0:
                        pprev = p_hist[sp - 1]
                        for qh in range(2):
                            qs = slice(qh * 512, (qh + 1) * 512)
                            nc.tensor.matmul(
                                pv_ps[:, qs],
                                vT2[:, h, sp - 1, :, 0:65].bitcast(F8),
                                pprev.bitcast(F8)[:, :, qs],
                                start=(sp == 1), stop=False,
                                perf_mode=DRM,
                            )
                for qh in range(2):
                    qs = slice(qh * 512, (qh + 1) * 512)
                    nc.tensor.matmul(
                        pv_ps[:, qs],
                        vT2[:, h, NSP - 1, :, 0:65].bitcast(F8),
                        p_hist[NSP - 1].bitcast(F8)[:, :, qs],
                        start=False, stop=True, perf_mode=DRM,
                    )
                # free the pv bank fast; Act computes 1/rowsum = exp(-ln)
                # straight from PSUM; the PE broadcast + DVE multiply run
                # inside the next head's pass
                lnr = rsp.tile([1, TS], F32, name="lnr", tag="lnr")
                nc.scalar.activation(lnr, pv_ps[64:65, :], AF.Ln)
                pvs = rsp.tile([65, TS], F32, name="pvs", tag="pvs")
                nc.vector.tensor_copy(pvs, pv_ps)
                recip = rsp.tile([1, TS], F32R, name="recip", tag="recip")
                nc.scalar.activation(recip, lnr, AF.Exp, scale=-1.0)
                pending_bc.append((h, pvs, recip))

            while pending_bc:
                norm_pe(*pending_bc.pop())

            # ---- proj (fp8 DoubleRow over head pairs) + residual ----
            out_sb = outp.tile([128, 8, C], F32, name="out_sb")
            for tci in range(8):
                pr_ps = psB.tile([128, C], F32, name="pr_ps", tag="vt")
                nc.tensor.matmul(
                    pr_ps, stack4[:, :, tci * 128:(tci + 1) * 128].bitcast(F8),
                    pT8.bitcast(F8), start=True, stop=True, perf_mode=DRM,
                )
                nc.vector.tensor_add(out_sb[:, tci, :], pr_ps, xT_sb[:, tci, :])
                eng = nc.sync if tci % 2 == 0 else nc.scalar
                eng.dma_start(
                    out=yT_d[tci * 128:(tci + 1) * 128, :], in_=out_sb[:, tci, :]
                )

    import bass_rust as _bass_rust
    _bass_rust.move_matmul_waits_to_ldweights(nc.m)
    _bass_rust.generate_event_semaphores(nc)
    return nc


def host_prep(inputs):
    """Per-core input dicts (slicing / transpose / dtype packing only)."""
    import ml_dtypes
    bf = ml_dtypes.bfloat16
    f8 = ml_dtypes.float8_e4m3

    x = np.ascontiguousarray(np.asarray(inputs["x"], np.float32).reshape(2, C, T))
    qkv_w = np.asarray(inputs["qkv_w"], np.float32)
    proj_w = np.asarray(inputs["proj_w"], np.float32)
    norm_w = np.ascontiguousarray(np.asarray(inputs["norm_w"], np.float32))
    norm_b = np.ascontiguousarray(np.asarray(inputs["norm_b"], np.float32))
    proj_b = np.ascontiguousarray(np.asarray(inputs["proj_b"], np.float32))

    q_idx = np.concatenate([np.arange(h * 192, h * 192 + 64) for h in range(H)])
    wqT = qkv_w[q_idx].T
    wkT = qkv_w[q_idx + 64].T
    wvT = qkv_w[q_idx + 128].T

    def dr_pack(wT):
        return np.ascontiguousarray(
            wT.reshape(2, 128, C).transpose(1, 0, 2).astype(f8)).view(np.uint8)

    pT8 = dr_pack(proj_w.T)

    sel = np.zeros((128, 16), np.float32)
    sel[np.arange(128), np.arange(128) // 8] = 1.0 / 8.0
    selN = sel / 4096.0
    expand = np.zeros((16, 128), np.float32)
    expand[np.arange(128) // 8, np.arange(128)] = 1.0

    shared = {
        "wq2": dr_pack(wqT), "wk2": dr_pack(wkT), "wv2": dr_pack(wvT),
        "pT8": pT8,
        "normw": np.ascontiguousarray(norm_w.reshape(2, 128, 1)),
        "normb": np.ascontiguousarray(norm_b.reshape(2, 128, 1)),
        "sel": sel, "selN": selN, "expand": expand,
        "ones": np.ones((128, 128), np.float32),
    }
    in_maps = []
    for core in range(8):
        b, i = core // 4, core % 4
        t0 = i * TS
        m = dict(shared)
        xr = np.roll(x[b], -t0, axis=1)
        m["xb"] = np.ascontiguousarray(xr.reshape(2, 128, T).transpose(1, 0, 2)
                                       ).astype(bf)
        m["xTb"] = np.ascontiguousarray(x[b, :, t0:t0 + TS].T
                                        + proj_b[None, :])
        in_maps.append(m)
    return in_maps


def gather(core_outs):
    y = np.empty((2, C, T), np.float32)
    for core in range(8):
        b, i = core // 4, core % 4
        y[b, :, i * TS:(i + 1) * TS] = core_outs[core].T
    return y.reshape(2, C, 16, 16, 16)


_NC = None


def _get_nc():
    global _NC
    if _NC is None:
        _NC = build_nc()
    return _NC


def run(inputs, trace=False, trace_cores=None):
    nc = _get_nc()
    in_maps = host_prep(inputs)
    res = run_bass_kernel_spmd(
        nc, in_maps, list(range(8)), trace=trace, trace_cores=trace_cores
    )
    out = gather([res.results[c]["yT"] for c in range(8)])
    return out, res


def kernel(**inputs) -> np.ndarray:
    out, _ = run(inputs)
    return out
